# revision 24
# baseline (speedup 1.0000x reference)
"""MBD degradation-imputation sampling step on 8 Trainium2 NeuronCores.

v3 — transposed + missing-packed layout.

Key observations:
  * Observed positions contribute nothing to the consistency scores once
    c0 saturates the clip there (v == q == 1/sigma, residual 0), and the
    weighted sum at observed positions is never read by the output
    (out = observed_data where mask). So only the ~30% missing (t,f)
    positions need to touch the device at all.
  * Host packs the missing positions' eps rows into epsT [TFm, NLOC]
    (positions major, padded to a multiple of 128). On-device tiles are
    [128 positions x 256 samples]; c0 / -q become per-partition [128,1]
    scalar APs; the score residual (v-q)^2 is ONE activation op
    (Square, bias=-q); per-sample score reduction (over positions =
    partitions) runs on the TensorEngine via ones-vector matmuls
    accumulating in PSUM; clipped candidates v are cached in SBUF as
    fp16 so eps is read from HBM exactly once; pass B is a v-cache
    sweep of scalar_tensor_tensor mult+accum against broadcast weights.
  * Output is the packed c1*weighted vector; the host scatters it into
    np.where(mask, observed, .).

`stage` truncates the program for hardware bisection:
  1 = pass A only, 2 = +AllGather/softmax, 3 = +pass B (no AllReduce),
  4 = full kernel.
"""

from contextlib import ExitStack

import numpy as np

import concourse.bass as bass
import concourse.tile as tile
from concourse import bacc, mybir
from concourse.bass_utils import run_bass_kernel_spmd

N_CORES = 8
N, T, F = 2048, 512, 64
P = 128
TF = T * F                      # 32768
NLOC = N // N_CORES             # 256 samples per core
TEMP = 0.1
T_STEPS = 1000
C0_OBS = 100.0                  # saturates the clip; fp16-safe

F32 = mybir.dt.float32
F16 = mybir.dt.float16
AX = mybir.AxisListType
ALU = mybir.AluOpType
ACTF = mybir.ActivationFunctionType


def _schedule_scalars(i: int):
    s = 0.008
    x = np.linspace(0, T_STEPS, T_STEPS + 1, dtype=np.float64)
    ac = np.cos((x / T_STEPS + s) / (1 + s) * np.pi * 0.5) ** 2
    ac = ac / ac[0]
    betas = np.clip(1.0 - ac[1:] / ac[:-1], 0.0, 0.999)
    alphas = 1.0 - betas
    acp = np.cumprod(alphas)
    abar_i = np.float32(acp[i])
    sigma_i = np.float32(np.sqrt(1.0 - acp[i]))
    alpha_i = np.float32(alphas[i])
    abar_im1 = np.float32(acp[i - 1])
    sa = np.float32(np.sqrt(abar_i))
    # the reference's Yi terms cancel exactly; out_missing = c1 * weighted
    c1 = np.float32(sa / np.float32(np.sqrt(alpha_i)) / np.float32(np.sqrt(abar_im1)))
    return sigma_i, c1


def _build(sigma_i: float, c1: float, ntm: int, stage: int = 4):
    inv_sig = float(np.float32(1.0 / np.float32(sigma_i)))
    sigma_i = float(np.float32(sigma_i))
    c1 = float(np.float32(c1))
    cA = float(np.float32(-(np.float32(sigma_i) ** 2) / np.float32(TF)))
    tfm = ntm * P

    nc = bacc.Bacc(
        "TRN2", target_bir_lowering=False, debug=False, num_devices=N_CORES
    )
    epsT_d = nc.dram_tensor("epsT", [tfm, NLOC], F32, kind="ExternalInput")
    c0_d = nc.dram_tensor("c0", [P, ntm], F32, kind="ExternalInput")
    negq_d = nc.dram_tensor("negq", [P, ntm], F32, kind="ExternalInput")
    out_d = nc.dram_tensor("out", [(ntm + 1) * P], F32, kind="ExternalOutput")

    wu_loc_d = nc.dram_tensor("wu_loc", [1], F32)
    wu_all_d = nc.dram_tensor("wu_all", [1], F32, addr_space="Shared")
    st_loc_d = nc.dram_tensor("st_loc", [2], F32)
    st_all_d = nc.dram_tensor("st_all", [2], F32, addr_space="Shared")
    nw = (ntm + 1) * P
    ws_loc_d = nc.dram_tensor("ws_loc", [nw], F32)
    ws_all_d = nc.dram_tensor("ws_all", [nw], F32, addr_space="Shared")

    rg = [list(range(N_CORES))]
    row1 = lambda d: d.ap().rearrange("(a n) -> a n", a=1)

    with tile.TileContext(nc) as tc, ExitStack() as ctx:
        epsT_ap = epsT_d.ap()

        cons = ctx.enter_context(tc.tile_pool(name="cons", bufs=1))
        epsp = ctx.enter_context(tc.tile_pool(name="epsp", bufs=8))
        work = ctx.enter_context(tc.tile_pool(name="work", bufs=6))
        zpool = ctx.enter_context(tc.tile_pool(name="zpool", bufs=8))
        smal = ctx.enter_context(tc.tile_pool(name="smal", bufs=1))
        psum = ctx.enter_context(tc.tile_pool(name="psum", bufs=1, space="PSUM"))

        # warm-up collective: absorbs the first-collective stream overhead
        # while pass A runs; the real stats AllReduce then starts promptly.
        nc.gpsimd.collective_compute(
            "AllReduce", ALU.add,
            ins=[wu_loc_d.ap()], outs=[wu_all_d.ap()], replica_groups=rg,
        )

        # ------------- persistent tiles -------------
        c0_t = cons.tile([P, ntm], F32, tag="c0", name="c0_t")
        nc.gpsimd.dma_start(out=c0_t[:], in_=c0_d.ap())
        nq_t = cons.tile([P, ntm], F32, tag="nq", name="nq_t")
        nc.gpsimd.dma_start(out=nq_t[:], in_=negq_d.ap())
        ones = cons.tile([P, 1], F16, tag="ones", name="ones")
        nc.gpsimd.memset(ones[:], 1.0)
        vcache = cons.tile([P, ntm * NLOC], F16, tag="vc", name="vcache")

        # ---------------- pass A ----------------
        ps = psum.tile([1, NLOC], F32, tag="score", name="ps")
        assert ntm % 2 == 0
        for tp_ in range(ntm // 2):
            e_t = epsp.tile([P, 2 * NLOC], F16, tag="eps", name="e_t")
            nc.gpsimd.dma_start(
                out=e_t[:],
                in_=epsT_ap[tp_ * 2 * P:(tp_ + 1) * 2 * P, :].rearrange(
                    "(p b) n -> p (b n)", b=2
                ),
            )
            for h in range(2):
                t = 2 * tp_ + h
                u_t = work.tile([P, NLOC], F16, tag="u", name="u_t")
                nc.vector.tensor_scalar(
                    out=u_t[:], in0=e_t[:, h * NLOC:(h + 1) * NLOC],
                    scalar1=c0_t[:, t:t + 1],
                    scalar2=inv_sig, op0=ALU.add, op1=ALU.min,
                )
                vsl = vcache[:, t * NLOC:(t + 1) * NLOC]
                nc.vector.tensor_scalar(
                    out=vsl, in0=u_t[:], scalar1=-inv_sig, scalar2=None,
                    op0=ALU.max,
                )
                z_t = zpool.tile([P, NLOC], F16, tag="z", name="z_t")
                nc.scalar.activation(
                    out=z_t[:], in_=vsl, func=ACTF.Square, bias=nq_t[:, t:t + 1],
                )
                nc.tensor.matmul(
                    ps[:], lhsT=ones[:], rhs=z_t[:],
                    start=(t == 0), stop=(t == ntm - 1),
                )

        # local score stats: s_loc = cA * colsum(z); pack [sum, sumsq]
        s_loc = smal.tile([1, NLOC], F32, tag="sloc", name="s_loc")
        nc.scalar.copy(s_loc[:], ps[:])
        nc.vector.tensor_scalar_mul(s_loc[:], s_loc[:], cA)
        ssum = smal.tile([1, 1], F32, tag="ssum", name="ssum")
        nc.vector.tensor_reduce(ssum[:], s_loc[:], axis=AX.X, op=ALU.add)
        sq = smal.tile([1, NLOC], F32, tag="sq", name="sq")
        ssq = smal.tile([1, 1], F32, tag="ssq", name="ssq")
        nc.scalar.activation(
            out=sq[:], in_=s_loc[:], func=ACTF.Square, accum_out=ssq[:],
        )
        pack = smal.tile([1, 2], F32, tag="pack", name="pack")
        nc.vector.tensor_copy(pack[:, 0:1], ssum[:])
        nc.vector.tensor_copy(pack[:, 1:2], ssq[:])
        nc.sync.dma_start(out=row1(st_loc_d), in_=pack[:])
        if stage <= 1:
            nc.sync.dma_start(
                out=out_d.ap()[0:2].rearrange("(a n) -> a n", a=1), in_=pack[:]
            )

        # ---------------- stats AllReduce + local exp weights ----------------
        if stage >= 2:
            nc.gpsimd.collective_compute(
                "AllReduce", ALU.add,
                ins=[st_loc_d.ap()], outs=[st_all_d.ap()], replica_groups=rg,
            )
            st = smal.tile([1, 2], F32, tag="st", name="st")
            nc.sync.dma_start(out=st[:], in_=row1(st_all_d))
            mean = smal.tile([1, 1], F32, tag="mean", name="mean")
            nc.vector.tensor_scalar_mul(mean[:], st[:, 0:1], 1.0 / N)
            m2 = smal.tile([1, 1], F32, tag="m2", name="m2")
            nc.vector.tensor_scalar_mul(m2[:], st[:, 1:2], 1.0 / N)
            msq = smal.tile([1, 1], F32, tag="msq", name="msq")
            nc.vector.tensor_tensor(msq[:], mean[:], mean[:], ALU.mult)
            var = smal.tile([1, 1], F32, tag="var", name="var")
            nc.vector.tensor_tensor(var[:], m2[:], msq[:], ALU.subtract)
            nc.vector.tensor_scalar_mul(var[:], var[:], N / (N - 1.0))
            std = smal.tile([1, 1], F32, tag="std", name="std")
            nc.scalar.activation(out=std[:], in_=var[:], func=ACTF.Sqrt)
            nc.vector.tensor_scalar_max(std[:], std[:], 1e-4)
            inv10 = smal.tile([1, 1], F32, tag="inv10", name="inv10")
            nc.vector.reciprocal(inv10[:], std[:])
            nc.vector.tensor_scalar_mul(inv10[:], inv10[:], 1.0 / TEMP)
            bg = smal.tile([1, 1], F32, tag="bg", name="bg")
            nc.vector.tensor_scalar(
                out=bg[:], in0=mean[:], scalar1=inv10[:], scalar2=-1.0,
                op0=ALU.mult, op1=ALU.mult,
            )
            # unnormalized weights e = exp((s - mean)/std/TEMP); Z deferred
            e_loc = smal.tile([1, NLOC], F32, tag="eloc", name="e_loc")
            nc.scalar.activation(
                out=e_loc[:], in_=s_loc[:], func=ACTF.Exp,
                scale=inv10[:], bias=bg[:],
            )
            zpart = smal.tile([1, 1], F32, tag="zpart", name="zpart")
            nc.vector.tensor_reduce(zpart[:], e_loc[:], axis=AX.X, op=ALU.add)
            wbc = cons.tile([P, NLOC], F32, tag="wbc", name="wbc")
            nc.gpsimd.partition_broadcast(wbc[:], e_loc[:])
            if stage <= 2:
                nc.sync.dma_start(
                    out=out_d.ap()[0:1].rearrange("(a n) -> a n", a=1),
                    in_=zpart[:],
                )

        # -------- pass B: unnormalized weighted sum + Z in last column --------
        if stage >= 3:
            w_out = cons.tile([P, ntm + 1], F32, tag="wout", name="w_out")
            nc.vector.memset(w_out[:, ntm:ntm + 1], 0.0)
            nc.vector.tensor_copy(w_out[0:1, ntm:ntm + 1], zpart[:])
            for t in range(ntm):
                scr = work.tile([P, NLOC], F32, tag="scr", name="scr")
                nc.vector.scalar_tensor_tensor(
                    out=scr[:], in0=vcache[:, t * NLOC:(t + 1) * NLOC],
                    scalar=1.0, in1=wbc[:], op0=ALU.mult, op1=ALU.mult,
                    accum_out=w_out[:, t:t + 1],
                )
            nc.sync.dma_start(
                out=ws_loc_d.ap().rearrange("(p c) -> p c", p=P), in_=w_out[:]
            )
            if stage <= 3:
                nc.sync.dma_start(
                    out=out_d.ap().rearrange("(p c) -> p c", p=P), in_=w_out[:]
                )

        # ------------- AllReduce; host does the final Z-normalization -------------
        if stage >= 4:
            nc.gpsimd.collective_compute(
                "AllReduce", ALU.add,
                ins=[ws_loc_d.ap()], outs=[ws_all_d.ap()], replica_groups=rg,
            )
            nc.sync.dma_start(out=out_d.ap(), in_=ws_all_d.ap())

    nc.compile()
    return nc


_CACHE: dict = {}
TRACE = False
STAGE = 4
LAST_RESULTS = None


def kernel(Xbar_i, observed_data, time_points, mask, eps, deg_a, deg_b, i):
    global LAST_RESULTS
    i = int(i)
    sigma_i, c1 = _schedule_scalars(i)

    inv_sig = np.float32(1.0) / sigma_i
    Xb = np.asarray(Xbar_i, np.float32)
    obs = np.asarray(observed_data, np.float32)
    msk = np.asarray(mask, bool)
    tp = np.asarray(time_points, np.float32)
    da = np.asarray(deg_a, np.float32)
    db = np.asarray(deg_b, np.float32)
    epsf = np.asarray(eps, np.float32)

    miss_idx = np.flatnonzero(~msk.reshape(-1))
    M = len(miss_idx)
    ntm = max(2, 2 * (-(-M // (2 * P))))   # even # of position tiles
    tfm = ntm * P

    key = ("v3", i, ntm, STAGE)
    if key not in _CACHE:
        _CACHE[key] = _build(float(sigma_i), float(c1), ntm, stage=STAGE)
    nc = _CACHE[key]

    pred = da[None, :] + db[None, :] * tp[:, None]
    c0_m = (Xb.reshape(-1)[miss_idx] * inv_sig).astype(np.float32)
    negq_m = (-(pred.reshape(-1)[miss_idx] * inv_sig)).astype(np.float32)
    c0_pk = np.full(tfm, np.float32(C0_OBS), np.float32)
    c0_pk[:M] = c0_m
    negq_pk = np.full(tfm, np.float32(-inv_sig), np.float32)
    negq_pk[:M] = negq_m
    # packed index r = t*128 + p  ->  device layout [p, t]
    pk = lambda a: np.ascontiguousarray(a.reshape(ntm, P).T)
    c0_pm = pk(c0_pk)
    negq_pm = pk(negq_pk)

    in_maps = []
    for c in range(N_CORES):
        shard = epsf[c * NLOC:(c + 1) * NLOC].reshape(NLOC, TF)
        epsT = np.zeros((tfm, NLOC), np.float32)
        epsT[:M] = shard[:, miss_idx].T
        # row order (tp, p, b): partition p's DMA line for a tile pair is
        # its two 1 KB chunks back to back (2 KB contiguous per partition)
        epsT = np.ascontiguousarray(
            epsT.reshape(ntm // 2, 2, P, NLOC).transpose(0, 2, 1, 3)
        ).reshape(tfm, NLOC)
        in_maps.append({"epsT": epsT, "c0": c0_pm, "negq": negq_pm})
    kr = run_bass_kernel_spmd(nc, in_maps, list(range(N_CORES)), trace=TRACE)
    LAST_RESULTS = kr
    res = kr.results[0]["out"].reshape(P, ntm + 1)
    zsum = res[0, ntm]
    scale = np.float32(c1) * sigma_i / zsum
    vals = np.ascontiguousarray(res[:, 0:ntm].T).reshape(tfm)[:M] * scale
    out_flat = np.where(msk.reshape(-1), obs.reshape(-1),
                        np.float32(0.0)).astype(np.float32)
    out_flat[miss_idx] = vals.astype(np.float32)
    return out_flat.reshape(T, F)
